# revision 1
# baseline (speedup 1.0000x reference)
"""Trainium2 Bass kernel for a 3-layer binary-weight MLP.

Problem (nn_MLP_56779467653689):
    x: [8192, 1024] f32
    h = relu(s0 * (x @ W0)) * 2      W0 = 2*k0-1  in {-1,+1}, [1024, 4096]
    h = relu(s1 * (h @ W1)) * 2      W1 [4096, 4096]
    out = s2 * (h @ W2)              W2 [4096, 1024]

Strategy: pure data-parallel over tokens across 8 NeuronCores (1024
tokens/core). Per core, activations live in SBUF as [features, tokens]
(features on partitions) so layers chain with no transposes. Weights are
pre-packed on the host into per-output-strip SBUF layout and streamed from
HBM. Matmuls run in bf16 (±1 weights are exact in bf16) with fp32 PSUM
accumulation; relu(s*acc)*2 == relu((2s)*acc) folds into one ACT per tile.
"""

from contextlib import ExitStack

import ml_dtypes
import numpy as np

P = 128
TOKENS = 8192
D_IN = 1024
D_H = 4096
D_OUT = 1024
N_CORES = 8
TOK_PER_CORE = TOKENS // N_CORES  # 1024
TOK_TILE = 512
NT = TOK_PER_CORE // TOK_TILE  # 2

BF16 = ml_dtypes.bfloat16

# Set TRACE=True (from test.py) to profile; LAST_EXEC_TIME_NS then holds the
# max per-core HW exec time of the most recent kernel() call.
TRACE = False
TRACE_CORES = None  # e.g. list(range(8)) to profile every core
LAST_EXEC_TIME_NS = None
LAST_RESULT = None

_cache = {}


def _dense_layer(nc, wpool, pspool, in_slice, w_dram, k_sub, n_t, evict,
                 t_outer=False):
    """out[n] strips = act(W[:, n-strip].T @ in) for n in range(n_t).

    in_slice(j, t): AP of the input block [P, TOK_TILE] for contraction
    tile j, token tile t. w_dram: packed [n_t, P, k_sub*P]. evict(n, t, ps)
    consumes the accumulated PSUM tile for (output strip n, token tile t).
    t_outer: each accumulation chain touches one token half, so layer 1's
    first chain starts after only the t=0 input halves landed.
    """
    import concourse.mybir as mybir

    for n in range(n_t):
        w = wpool.tile([P, k_sub * P], mybir.dt.bfloat16, tag="w", name=f"w_{n}")
        nc.sync.dma_start(out=w[:], in_=w_dram[n])
        if t_outer:
            for t in range(NT):
                ps = pspool.tile(
                    [P, TOK_TILE], mybir.dt.float32, tag="ps", name=f"ps_{n}_{t}"
                )
                for j in range(k_sub):
                    nc.tensor.matmul(
                        ps[:],
                        w[:, j * P : (j + 1) * P],
                        in_slice(j, t),
                        start=(j == 0),
                        stop=(j == k_sub - 1),
                    )
                evict(n, t, ps)
        else:
            # t-inner: consecutive matmuls alternate PSUM banks, which
            # measures ~0.7 ns/MM faster than same-bank accumulation runs.
            pss = [
                pspool.tile(
                    [P, TOK_TILE], mybir.dt.float32, tag="ps", name=f"ps_{n}_{t}"
                )
                for t in range(NT)
            ]
            for j in range(k_sub):
                for t in range(NT):
                    nc.tensor.matmul(
                        pss[t][:],
                        w[:, j * P : (j + 1) * P],
                        in_slice(j, t),
                        start=(j == 0),
                        stop=(j == k_sub - 1),
                    )
            for t in range(NT):
                evict(n, t, pss[t])


def _prune_dma_waits(nc, max_waits=1):
    """Drop transitively-implied waits from DMA instructions.

    DMA queue-entry descriptors hold a single sync wait; Tile's sem
    assignment is per-proc minimal but not transitively minimal across
    procs, so a recycled SBUF slot's DMA can carry WAR (engine) + WAW
    (prev slot writer's DMA lane) + lane-recycle waits = 3. The WAW (and
    often the recycle) wait is implied by the engine wait: the readers
    counted by the WAR threshold themselves waited on those DMAs.

    Soundness: a wait (s >= v) on instruction I is dropped only when the
    completion clocks implied by I's *other* waits already guarantee
    cumulative increments of s reached v. Completion clocks are built
    forward over the scheduled BIR order giving same-stream predecessor
    credit only to in-order engines (PE/ACT/DVE/SP), never to DMA lanes
    or Pool. Unrecognized wait/update modes contribute no credit, so
    unknowns can only inhibit pruning, never enable it.
    """
    import bisect

    import bass_rust

    IN_ORDER_ENGINES = {
        "EngineType.PE",
        "EngineType.Activation",
        "EngineType.DVE",
        "EngineType.SP",
    }

    sem_hist = {}  # sem -> ([cumulative values], [clocks at completion])
    sem_cum = {}  # sem -> cumulative increments so far
    eng_clock = {}  # engine -> completion clock of last instruction
    poisoned = set()  # sems with non-monotonic updates: no credit

    def cc(sem, val):
        """Completion clock implied by observing sem >= val, or None."""
        if sem in poisoned:
            return None
        hist = sem_hist.get(sem)
        if not hist or hist[0][-1] < val:
            return None
        return hist[1][bisect.bisect_left(hist[0], val)]

    def merge(dst, src):
        for k, v in src.items():
            if dst.get(k, 0) < v:
                dst[k] = v

    pruned = 0
    for bb in nc.m.functions[0].blocks:
        for inst in bb.instructions:
            si = inst.sync_info
            waits = list(si.on_wait or []) if si is not None else []
            ups = list(si.on_update or []) if si is not None else []
            is_dma = type(inst).__name__ == "InstDMACopy"

            clock = {}
            if not is_dma:
                prev = eng_clock.get(str(inst.engine))
                if prev is not None and str(inst.engine) in IN_ORDER_ENGINES:
                    merge(clock, prev)
            for w in waits:
                if w.wait_mode == "sem-ge-imm" and w.wait_value is not None:
                    c = cc(w.ant_name, w.wait_value)
                    if c is not None:
                        merge(clock, c)

            # Per-encoding wait budgets: DMA queue entries hold 1 wait;
            # engine instructions hold 2. Drain/EventSemaphore/control flow
            # are lowered specially by walrus — leave them alone.
            tname = type(inst).__name__
            if is_dma:
                cap = max_waits
            elif tname in ("InstDrain", "InstEventSemaphore", "InstCall",
                           "InstUnconditionalBranch", "InstISA"):
                cap = None
            else:
                cap = 2

            if cap is not None and len(waits) > cap:
                kept = list(waits)
                changed = True
                while len(kept) > cap and changed:
                    changed = False
                    for w in list(kept):
                        if w.wait_mode != "sem-ge-imm" or w.wait_value is None:
                            continue
                        implied = {}
                        provable = True
                        for o in kept:
                            if o is w:
                                continue
                            if o.wait_mode != "sem-ge-imm" or o.wait_value is None:
                                provable = False
                                break
                            c = cc(o.ant_name, o.wait_value)
                            if c is None:
                                provable = False
                                break
                            merge(implied, c)
                        if provable and implied.get(w.ant_name, 0) >= w.wait_value:
                            kept.remove(w)
                            pruned += 1
                            changed = True
                            break
                # Anything still over budget is left for Bacc's
                # generate_event_semaphores pass to split legally.
                if len(kept) != len(waits):
                    inst.sync_info = bass_rust.SyncInfo(on_wait=kept, on_update=ups)

            own = {}
            for u in ups:
                if u.update_mode not in ("sem-inc", "sem-add-imm"):
                    poisoned.add(u.ant_name)
                    continue
                inc = 1 if u.update_mode == "sem-inc" else u.update_value
                if inc is None:
                    poisoned.add(u.ant_name)
                    continue
                sem = u.ant_name
                sem_cum[sem] = sem_cum.get(sem, 0) + inc
                own[sem] = sem_cum[sem]
            merge(clock, own)
            for sem, cum in own.items():
                vals, clocks = sem_hist.setdefault(sem, ([], []))
                vals.append(cum)
                clocks.append(clock)
            if not is_dma:
                eng_clock[str(inst.engine)] = clock
    return pruned


def _build(a0, a1, a2):
    """Build the SPMD single-core program (same NEFF on all 8 cores)."""
    import concourse.mybir as mybir
    import concourse.tile as tile
    from concourse import bacc

    # Bacc (not plain Bass): its finalize() runs the wait-legalization
    # passes (move_matmul_waits_to_ldweights, generate_event_semaphores)
    # that split multi-wait instructions to the 1-wait HW encoding.
    nc = bacc.Bacc(
        "TRN2",
        target_bir_lowering=False,
        debug=False,
        enable_asserts=False,
        num_devices=N_CORES,
    )
    bf = mybir.dt.bfloat16
    f32 = mybir.dt.float32

    xt = nc.dram_tensor("xt", [D_IN, TOK_PER_CORE], bf, kind="ExternalInput")
    w0p = nc.dram_tensor("w0p", [D_H // P, P, D_IN], bf, kind="ExternalInput")
    w1p = nc.dram_tensor("w1p", [D_H // P, P, D_H], bf, kind="ExternalInput")
    w2p = nc.dram_tensor("w2p", [D_OUT // P, P, D_H], bf, kind="ExternalInput")
    outt = nc.dram_tensor("outt", [D_OUT, TOK_PER_CORE], f32, kind="ExternalOutput")

    relu = mybir.ActivationFunctionType.Relu

    with tile.TileContext(nc) as tc, ExitStack() as ctx:
        xpool = ctx.enter_context(tc.tile_pool(name="xp", bufs=1))
        h1pool = ctx.enter_context(tc.tile_pool(name="h1p", bufs=1))
        h2pool = ctx.enter_context(tc.tile_pool(name="h2p", bufs=1))
        wpool = ctx.enter_context(tc.tile_pool(name="wp", bufs=4))
        opool = ctx.enter_context(tc.tile_pool(name="op", bufs=3))
        pspool = ctx.enter_context(tc.tile_pool(name="psp", bufs=8, space="PSUM"))

        # x as per-j half-tiles in consumption order (t=0 first): the first
        # accumulation chain starts after just x_0_0 (128 KB) + one weight
        # strip, with later tiles streaming in behind the compute.
        x_half = [[None] * NT for _ in range(D_IN // P)]
        for t in range(NT):
            for j in range(D_IN // P):
                h = xpool.tile([P, TOK_TILE], bf, tag=f"x{j}_{t}", name=f"x_{j}_{t}")
                # ACT HWDGE queue: runs in parallel with the weight stream
                # on the SP queue, shortening the DMA-bound startup.
                nc.scalar.dma_start(
                    out=h[:],
                    in_=xt[j * P : (j + 1) * P, t * TOK_TILE : (t + 1) * TOK_TILE],
                )
                x_half[j][t] = h

        def x_slice(j, t):
            return x_half[j][t][:]

        h1_tiles = [
            h1pool.tile([P, TOK_PER_CORE], bf, tag=f"h1_{n}", name=f"h1_{n}")
            for n in range(D_H // P)
        ]
        h2_tiles = [
            h2pool.tile([P, TOK_PER_CORE], bf, tag=f"h2_{n}", name=f"h2_{n}")
            for n in range(D_H // P)
        ]

        def evict_h(h_tiles, scale):
            def evict(n, t, ps):
                nc.scalar.activation(
                    h_tiles[n][:, t * TOK_TILE : (t + 1) * TOK_TILE],
                    ps[:],
                    relu,
                    scale=scale,
                )

            return evict

        def evict_out(n, t, ps):
            # Stream each token half out as soon as its eviction lands —
            # the final strip's DMA starts one eviction earlier. The two
            # halves evict on different engines (ACT / DVE) so the last
            # strip's evictions run in parallel instead of serializing on
            # ScalarE right before the final DMA.
            o = opool.tile([P, TOK_TILE], f32, tag="o", name=f"o_{n}_{t}")
            if t % 2 == 0:
                nc.scalar.mul(o[:], ps[:], a2)
            else:
                nc.vector.tensor_scalar_mul(o[:], ps[:], a2)
            nc.scalar.dma_start(
                out=outt[n * P : (n + 1) * P, t * TOK_TILE : (t + 1) * TOK_TILE],
                in_=o[:],
            )

        def h_slice(h_tiles):
            return lambda j, t: h_tiles[j][:, t * TOK_TILE : (t + 1) * TOK_TILE]

        _dense_layer(nc, wpool, pspool, x_slice, w0p,
                     D_IN // P, D_H // P, evict_h(h1_tiles, a0), t_outer=True)
        _dense_layer(nc, wpool, pspool, h_slice(h1_tiles), w1p,
                     D_H // P, D_H // P, evict_h(h2_tiles, a1))
        _dense_layer(nc, wpool, pspool, h_slice(h2_tiles), w2p,
                     D_H // P, D_OUT // P, evict_out)

    _prune_dma_waits(nc)
    nc.finalize()
    return nc


def _pack_w(k):
    """Bool [K, N] -> bf16 ±1 packed [N/P, P, K]: strip n, partition p,
    free j*P+c  <-  W[j*P+p, n*P+c] (partition = contraction for lhsT)."""
    K, N = k.shape
    w = np.where(k, np.float32(1.0), np.float32(-1.0)).astype(BF16)
    return np.ascontiguousarray(
        w.reshape(K // P, P, N // P, P).transpose(2, 1, 0, 3).reshape(N // P, P, K)
    )


def _enable_ntff_trace():
    """Best-effort plumbing for trace=True under axon in this image.

    The image's ``antenv`` lacks the ``axon_hooks`` shim that
    ``trn_agent_boot`` would normally register the NTFF profile hook
    into, and there is no artifact bucket — stub both.
    """
    import sys
    import types

    import concourse.bass_utils as bu

    bu.upload_artifacts = lambda tmpdir: tmpdir
    try:
        from antenv import axon_hooks
    except ImportError:
        import antenv

        axon_hooks = types.ModuleType("antenv.axon_hooks")
        _state = {"hook": None}
        axon_hooks.set_axon_ntff_profile_hook = lambda h: _state.__setitem__(
            "hook", h
        )
        axon_hooks.get_axon_ntff_profile_hook = lambda: _state["hook"]
        sys.modules["antenv.axon_hooks"] = axon_hooks
        antenv.axon_hooks = axon_hooks
    if axon_hooks.get_axon_ntff_profile_hook() is None:
        from trn_agent_boot.trn_boot import _ntff_profile_via_ctypes

        axon_hooks.set_axon_ntff_profile_hook(
            _ntff_profile_via_ctypes("/opt/axon/libaxon_pjrt.so")
        )


def kernel(x, k0, k1, k2, s0, s1, s2):
    global LAST_EXEC_TIME_NS, LAST_RESULT
    from concourse.bass_utils import run_bass_kernel_spmd

    if TRACE:
        _enable_ntff_trace()

    x = np.asarray(x)
    a0 = 2.0 * float(np.asarray(s0))
    a1 = 2.0 * float(np.asarray(s1))
    a2 = float(np.asarray(s2))

    key = (a0, a1, a2)
    if key not in _cache:
        _cache[key] = _build(a0, a1, a2)
    nc = _cache[key]

    w0p = _pack_w(np.asarray(k0))
    w1p = _pack_w(np.asarray(k1))
    w2p = _pack_w(np.asarray(k2))

    in_maps = []
    for i in range(N_CORES):
        xs = x[i * TOK_PER_CORE : (i + 1) * TOK_PER_CORE].astype(BF16)
        in_maps.append(
            {
                "xt": np.ascontiguousarray(xs.T),
                "w0p": w0p,
                "w1p": w1p,
                "w2p": w2p,
            }
        )

    res = run_bass_kernel_spmd(
        nc, in_maps, list(range(N_CORES)), trace=TRACE, trace_cores=TRACE_CORES
    )
    LAST_EXEC_TIME_NS = res.exec_time_ns
    LAST_RESULT = res
    out = np.concatenate(
        [res.results[i]["outt"].T for i in range(N_CORES)], axis=0
    )
    return np.ascontiguousarray(out)

